# revision 10
# baseline (speedup 1.0000x reference)
"""EnhancedTemporalAttention Trainium2 kernel (v3, fp8 DoubleRow).

Full module: GroupNorm(32) -> QKV 1x1conv -> 8-head attention (softmax) ->
out 1x1conv + bias -> +residual, on x [4, 512, 2048] fp32.

Sharding: 8 cores = (batch b = core//2) x (head-half hg = core%2).  Each
core computes GroupNorm stats + its 4 heads over the full sequence and a
partial out-projection (contraction over its 256 channels); the host sums
the two partials per batch and adds residual + b_out in fp32.

All heavy matmuls run in fp8 with the DoubleRow perf mode (2 contraction
rows per PE cell, 0.5 cycles/row):
  - QKV: x and (GroupNorm-folded) weights in e4m3, channel tiles paired as
    DR slots.
  - Scores: q/k stored as [h*32+d%32, d//32-slot, n] e4m3 so each head's
    64-dim contraction is 32 partitions x 2 slots.
  - AV: e^T stationary with CONSECUTIVE KEY BLOCKS as the two DR slots
    (256 keys per matmul); v carries a 1/32 ones column so softmax
    denominators ride the same PSUM tile (attn out is scaled x32, undone
    on the host).
  - Out-projection: attnout (e4m3) with the two head-pairs as DR slots.

exp splits three ways: ACT computes exact exp (scale 1/8, bias -SHIFT to
keep e4m3 finite) straight to e4m3; DVE and GPSIMD run an int8 Schraudolph
(i8 = s*A + B bitcast e5m2).  The softmax denominator uses fp32 PSUM, so
the shift cancels exactly.

PSUM: 3-slot ring of [128,1024] score tiles + one [128,8,128] AV
accumulator = 16KB exactly; transposes / out-proj / GN matvecs borrow
ring slots via half-slot views.
"""
import sys

sys.path.insert(0, "/opt/trn_rl_repo")

import numpy as np
import ml_dtypes

import concourse.bacc as bacc
import concourse.bass as bass
import concourse.tile as tile
from concourse import mybir
from concourse.bass_utils import run_bass_kernel_spmd

F32 = mybir.dt.float32
F32R = mybir.dt.float32r
BF16 = mybir.dt.bfloat16
I16 = mybir.dt.int16
I8 = mybir.dt.int8
E4 = mybir.dt.float8e4
E5 = mybir.dt.float8e5

B = 4
C = 512
N = 2048
H = 8
HL = 4             # local heads per core
D = 64
G = 32             # groupnorm groups
CPG = C // G       # 16 channels per group
UN = 4             # input-channel units of 128 (u = t*2 + slot)
EPS = 1e-4
SCALE = D ** -0.5
NKB = N // 128     # 16 key blocks
NJP = NKB // 2     # 8 key-block pairs (DR slots)
QC = 4             # query chunks of 512
AVS = 32.0         # attnout scale (ones col = 1/AVS, host divides)
SHIFT = 2.2        # exp(l - SHIFT): keeps e4m3 finite; cancels in softmax
AF = mybir.ActivationFunctionType
ALU = mybir.AluOpType
PM = mybir.MatmulPerfMode

# Schraudolph exp into e5m2 bits: i8 = s*A5 + B5, bitcast -> fp8e5
A5 = (4.0 / np.log(2.0)) * SCALE
B5 = 60.25 - (4.0 / np.log(2.0)) * SHIFT

# exp engine schedule: GPSIMD cannot touch PSUM, so exp runs on ACT + DVE
# only.  ACT units get exact exp -> e4m3; DVE units run the e5m2
# Schraudolph.  ACT is faster per element AND carries less misc load, so
# it takes the bigger share.
ACT_UNITS = 39     # of 64 (qc, jp, hp) dtype-units; rest on DVE


def _exp_engine_table():
    """(qc, jp, hp) -> ('act','act') | ('dve','dve'), evenly interleaved."""
    units = [(qc, jp, hp) for qc in range(QC) for jp in range(NJP)
             for hp in range(2)]
    table = {}
    acc = 0.0
    for u in units:
        acc += ACT_UNITS / 64.0
        if acc >= 1.0:
            table[u] = ("act", "act")
            acc -= 1.0
        else:
            table[u] = ("dve", "dve")
    return table


EXP_ENG = _exp_engine_table()


def _build():
    nc = bacc.Bacc("TRN2", target_bir_lowering=False, debug=False)
    x_in = nc.dram_tensor("x8", [128, UN, N], E4, kind="ExternalInput").ap()
    wbf_in = nc.dram_tensor("wbf", [128, UN, 768], BF16,
                            kind="ExternalInput").ap()
    wout_in = nc.dram_tensor("wout8", [128, 2, C], E4,
                             kind="ExternalInput").ap()
    gbo_in = nc.dram_tensor("gbo", [128, 8], F32, kind="ExternalInput").ap()
    gblk_in = nc.dram_tensor("gblk", [128, 8], F32, kind="ExternalInput").ap()
    gbt_in = nc.dram_tensor("gbt", [8, 128], F32, kind="ExternalInput").ap()
    id_in = nc.dram_tensor("ident", [128, 128], BF16,
                           kind="ExternalInput").ap()
    y_out = nc.dram_tensor("y", [C, N], F32, kind="ExternalOutput").ap()

    from contextlib import ExitStack
    with tile.TileContext(nc) as tc, ExitStack() as ctx:
        persist = ctx.enter_context(tc.tile_pool(name="persist", bufs=1))
        gn = ctx.enter_context(tc.tile_pool(name="gn", bufs=1))
        pspool = ctx.enter_context(tc.tile_pool(name="ps", bufs=1,
                                                space="PSUM"))
        expp = ctx.enter_context(tc.tile_pool(name="expp", bufs=1))
        drp = ctx.enter_context(tc.tile_pool(name="drp", bufs=1))

        # ---- persistent tiles ----
        x8 = persist.tile([128, UN, N], E4, tag="x8", name="x8")
        wbf = persist.tile([128, UN, 768], BF16, tag="wbf", name="wbf")
        w8 = persist.tile([128, UN, 768], E4, tag="w8", name="w8")
        w8out = persist.tile([128, 2, C], E4, tag="w8out", name="w8out")
        q8 = persist.tile([128, 2, N], E4, tag="q8", name="q8")
        k8 = persist.tile([128, 2, N], E4, tag="k8", name="k8")
        v2 = [persist.tile([128, 2, HL, 66], E4, tag=f"v2_{jp}",
                           name=f"v2_{jp}") for jp in range(NJP)]
        ident = persist.tile([128, 128], BF16, tag="ident", name="ident")

        # PSUM: S ring 3x[128,1024] (4KB each) + av [128,8,128] (4KB)
        def new_S():
            return pspool.tile([128, 1024], F32, tag="S", name="S", bufs=3)

        def new_ops():
            return new_S()[:, 0:512]

        av = pspool.tile([128, 8, 128], F32, tag="av", name="av", bufs=1)

        # ---- input loads ----
        for u in range(UN):
            nc.sync.dma_start(out=x8[:, u, :], in_=x_in[:, u, :])
        gbo4 = gn.tile([128, 8], F32, tag="gbo4")
        gblk = gn.tile([128, 8], F32R, tag="gblk")
        gbt = gn.tile([8, 128], F32R, tag="gbt")
        nc.sync.dma_start(out=gbo4, in_=gbo_in)
        nc.sync.dma_start(out=gblk, in_=gblk_in.bitcast(F32R))
        nc.sync.dma_start(out=gbt, in_=gbt_in.bitcast(F32R))
        for u in range(UN):
            nc.sync.dma_start(out=wbf[:, u, :], in_=wbf_in[:, u, :])
        nc.sync.dma_start(out=w8out.rearrange("p m c -> p (m c)"),
                          in_=wout_in.rearrange("p m c -> p (m c)"))
        nc.sync.dma_start(out=ident, in_=id_in)
        gbo = [gbo4[:, 2 * u:2 * u + 2] for u in range(UN)]

        # ---- GroupNorm stats (per 128-channel unit u) ----
        shift_t = gn.tile([128, 1], F32, tag="shift_t")
        nc.gpsimd.memset(shift_t, -SHIFT)
        eps_t = gn.tile([G, 1], F32, tag="eps_t")
        nc.vector.memset(eps_t, EPS)
        sqw = gn.tile([G, 1], F32, tag="sqw")
        nc.scalar.activation(out=sqw, in_=eps_t, func=AF.Sqrt)
        mvv = []
        for u in range(UN):
            stats = gn.tile([128, 4, 6], F32, tag=f"st{u}", name=f"st{u}")
            for sg in range(4):
                nc.vector.bn_stats(out=stats[:, sg, :],
                                   in_=x8[:, u, sg * 512:(sg + 1) * 512])
            mv = gn.tile([128, 2], F32, tag=f"mv{u}", name=f"mv{u}")
            nc.vector.bn_aggr(out=mv, in_=stats)
            mt = gn.tile([128, 2], F32R, tag=f"mvv{u}", name=f"mvv{u}")
            nc.vector.tensor_copy(mt[:, 0:1], mv[:, 0:1])
            nc.vector.scalar_tensor_tensor(
                out=mt[:, 1:2], in0=mv[:, 0:1], scalar=mv[:, 0:1],
                in1=mv[:, 1:2], op0=ALU.mult, op1=ALU.add)
            mvv.append(mt)
        g8ps = new_ops()
        for u in range(UN):
            nc.tensor.matmul(g8ps[0:8, u * 2:(u + 1) * 2],
                             lhsT=gblk, rhs=mvv[u],
                             start=(u == 0), stop=(u == UN - 1),
                             skip_group_check=True)
        g8 = gn.tile([8, UN, 2], F32, tag="g8")
        nc.vector.tensor_copy(g8.rearrange("p t s -> p (t s)"),
                              g8ps[0:8, 0:8])
        mean8 = gn.tile([8, UN], F32, tag="mean8")
        nc.vector.tensor_scalar_mul(mean8, g8[:, :, 0], 1.0 / CPG)
        ex28 = gn.tile([8, UN], F32, tag="ex28")
        nc.vector.tensor_scalar_mul(ex28, g8[:, :, 1], 1.0 / CPG)
        msq8 = gn.tile([8, UN], F32, tag="msq8")
        nc.vector.tensor_mul(msq8, mean8, mean8)
        var8 = gn.tile([8, UN], F32, tag="var8")
        nc.vector.tensor_tensor(out=var8, in0=ex28, in1=msq8,
                                op=ALU.subtract)
        std8 = gn.tile([8, UN], F32, tag="std8")
        nc.scalar.activation(out=std8, in_=var8, func=AF.Sqrt,
                             bias=eps_t[0:8, :])
        rstd8 = gn.tile([8, UN], F32, tag="rstd8")
        nc.vector.reciprocal(rstd8, std8)
        # preload the Exp table; chained after the real Sqrt via std8
        warm = gn.tile([8, UN], F32, tag="warm")
        nc.scalar.activation(out=warm, in_=std8, func=AF.Exp)
        mr8 = gn.tile([8, UN, 2], F32R, tag="mr8")
        nc.vector.tensor_copy(mr8[:, :, 0:1],
                              mean8.rearrange("p (t o) -> p t o", o=1))
        nc.vector.tensor_copy(mr8[:, :, 1:2],
                              rstd8.rearrange("p (t o) -> p t o", o=1))
        msps = new_ops()
        for u in range(UN):
            nc.tensor.matmul(msps[:, u * 2:(u + 1) * 2],
                             lhsT=gbt, rhs=mr8[:, u, :],
                             start=(u == 0), stop=(u == UN - 1),
                             skip_group_check=True)
        msb = gn.tile([128, UN, 2], F32, tag="msb")
        nc.vector.tensor_copy(msb.rearrange("p t s -> p (t s)"),
                              msps[:, 0:2 * UN])

        # per-channel scale_c = rstd*gamma; bias bb = beta - mean*scale_c
        qkvb_ps = new_ops()
        scale_c = []
        for u in range(UN):
            eng = nc.vector if u % 2 == 0 else nc.gpsimd
            sc = gn.tile([128, 1], F32, tag=f"sc{u}", name=f"sc{u}")
            eng.tensor_mul(sc, msb[:, u, 1:2], gbo[u][:, 0:1])
            scale_c.append(sc)
            tmp = gn.tile([128, 1], F32, tag=f"tmp{u}", name=f"tmp{u}")
            eng.tensor_mul(tmp, msb[:, u, 0:1], sc)
            bb = gn.tile([128, 1], BF16, tag=f"bb{u}", name=f"bb{u}")
            eng.tensor_tensor(out=bb, in0=gbo[u][:, 1:2], in1=tmp,
                              op=ALU.subtract)
            # qkv bias matvec against RAW weights, accumulated over u
            for oc in range(6):
                nc.tensor.matmul(qkvb_ps[:, oc:oc + 1],
                                 lhsT=wbf[:, u, oc * 128:(oc + 1) * 128],
                                 rhs=bb, start=(u == 0), stop=(u == UN - 1),
                                 skip_group_check=True)
        qkvb = gn.tile([128, 6], F32, tag="qkvb")
        nc.vector.tensor_copy(qkvb, qkvb_ps[:, 0:6])
        # v bias rides the x32-scaled attnout
        nc.vector.tensor_scalar_mul(qkvb[:, 4:6], qkvb[:, 4:6], AVS)

        # quantize folded weights to e4m3 (w8 = wbf * scale_c)
        for u in range(UN):
            eng = (nc.vector, nc.gpsimd, nc.vector, nc.gpsimd)[u]
            eng.tensor_scalar(out=w8[:, u, :], in0=wbf[:, u, :],
                              scalar1=scale_c[u], scalar2=None,
                              op0=ALU.mult)

        # ---- projections ----
        # PSUM-reading copies can only run on ACT or DVE (GPSIMD has no
        # PSUM access).
        copy_rr = [nc.scalar, nc.vector]

        def kq_proj(which, s, ncx, engi):
            """q/k for d-half slot s, query chunk ncx -> q8/k8[:, s, cols]."""
            ps = new_ops()
            col0 = (s if which == "q" else 2 + s) * 128
            for t in range(2):
                nc.tensor.matmul(
                    ps, lhsT=w8[:, 2 * t:2 * t + 2, col0:col0 + 128],
                    rhs=x8[:, 2 * t:2 * t + 2, ncx * 512:(ncx + 1) * 512],
                    start=(t == 0), stop=(t == 1), perf_mode=PM.DoubleRow)
            dst = (q8 if which == "q" else k8)[:, s, ncx * 512:(ncx + 1) * 512]
            boff = (0 if which == "q" else 2) + s
            eng = copy_rr[engi % 2]
            if eng is nc.scalar:
                nc.scalar.activation(out=dst, in_=ps, func=AF.Identity,
                                     bias=qkvb[:, boff:boff + 1])
            else:
                eng.tensor_scalar(out=dst, in0=ps,
                                  scalar1=qkvb[:, boff:boff + 1],
                                  scalar2=None, op0=ALU.add)

        def v_proj(nb, engi):
            """v for key block nb -> v2[nb//2][:, nb%2, :, 0:64]."""
            ps = new_ops()
            for t in range(2):
                nc.tensor.matmul(
                    ps[:, 0:256],
                    lhsT=x8[:, 2 * t:2 * t + 2, nb * 128:(nb + 1) * 128],
                    rhs=w8[:, 2 * t:2 * t + 2, 512:768],
                    start=(t == 0), stop=(t == 1), perf_mode=PM.DoubleRow)
            src = ps[:, 0:256].rearrange("p (h d) -> p h d", h=HL)
            dst = v2[nb // 2][:, nb % 2, :, 0:64]
            eng = copy_rr[engi % 2]
            if eng is nc.scalar:
                nc.scalar.activation(out=dst, in_=src, func=AF.Copy)
            else:
                eng.tensor_copy(dst, src)
            nc.gpsimd.memset(v2[nb // 2][:, nb % 2, :, 64:65], 1.0 / AVS)

        for s in range(2):
            for ncx in range(4):
                kq_proj("k", s, ncx, s * 4 + ncx)
        # q chunk 0 now so the attention stream can start
        kq_proj("q", 0, 0, 0)
        kq_proj("q", 1, 0, 1)

        # ---- attention stream ----
        # eT2[(qc, jp)]: [128 keys, 2 j-slots, 2048 (4h x 512q)] int8 ring
        eT2 = {}

        def get_eT2(qc, jp):
            if (qc, jp) not in eT2:
                eT2[(qc, jp)] = expp.tile([128, 2, N], I8, tag="eT2",
                                          name="eT2", bufs=12)
            return eT2[(qc, jp)]

        def emit_scores_exp(qc, j, hp):
            s = new_S()
            for hh in range(2):
                h = hp * 2 + hh
                nc.tensor.matmul(
                    s[:, hh * 512:(hh + 1) * 512],
                    lhsT=k8[h * 32:(h + 1) * 32, :, j * 128:(j + 1) * 128],
                    rhs=q8[h * 32:(h + 1) * 32, :, qc * 512:(qc + 1) * 512],
                    start=True, stop=True, perf_mode=PM.DoubleRow,
                    tile_position=(h * 32, 0), skip_group_check=True)
            et = get_eT2(qc, j // 2)
            dst = et[:, j % 2, hp * 1024:(hp + 1) * 1024]
            eng = EXP_ENG[(qc, j // 2, hp)][j % 2]
            if eng == "act":
                nc.scalar.activation(out=dst.bitcast(E4), in_=s,
                                     func=AF.Exp, scale=SCALE, bias=shift_t)
            else:
                nc.vector.tensor_scalar(out=dst, in0=s, scalar1=A5,
                                        scalar2=B5, op0=ALU.mult,
                                        op1=ALU.add)

        def emit_av(qc, m, jp):
            """8 DR matmuls: 256 keys (j-slot pair) x [128q, 65]."""
            et = get_eT2(qc, jp)
            dt = E4 if EXP_ENG[(qc, jp, m)][0] == "act" else E5
            eb = et.bitcast(dt)
            for qb in range(4):
                for hh in range(2):
                    c0 = m * 1024 + hh * 512 + qb * 128
                    nc.tensor.matmul(
                        av[:, qb * 2 + hh, 0:65],
                        lhsT=eb[:, :, c0:c0 + 128],
                        rhs=v2[jp][:, :, 2 * m + hh, 0:65],
                        start=(jp == 0), stop=(jp == NJP - 1),
                        perf_mode=PM.DoubleRow, skip_group_check=True)
            if jp == NJP - 1:
                drain_a(qc, m)

        avn_pend = {}

        def drain_a(qc, m):
            """rden + normalize av -> avn bf16 [128, 4qb, 2hh*64d]."""
            rden = drp.tile([128, 8, 1], F32, tag="rden", name="rden",
                            bufs=2)
            nc.vector.reciprocal(rden, av[:, :, 64:65])
            avn = drp.tile([128, 4, 128], BF16, tag="avn", name="avn",
                           bufs=2)
            avv = avn.rearrange("p qb c -> p (qb c)").rearrange(
                "p (qb h d) -> p qb h d", qb=4, h=2)
            s1 = rden.ap[1][0]
            for half in range(2):
                rb = bass.AP(tensor=rden.tensor,
                             offset=rden.offset + half * 4 * s1,
                             ap=[rden.ap[0], [s1 * 2, 2], [s1, 2], [0, 64]])
                nc.vector.tensor_tensor(
                    out=avv[:, 2 * half:2 * half + 2, :, :],
                    in0=av[:, 4 * half:4 * half + 4, 0:64].rearrange(
                        "p (qb h) d -> p qb h d", qb=2),
                    in1=rb, op=ALU.mult)
            avn_pend[(qc, m)] = avn

        op_tiles = {}

        def drain_b(qc, m, half, engi):
            """transpose 2 qb + bias-add/quantize -> op[:, m, half*256:]."""
            avn = avn_pend[(qc, m)]
            if qc not in op_tiles:
                op_tiles[qc] = drp.tile([128, 2, 512], E4, tag="op",
                                        name="op", bufs=2)
            tps_flat = new_ops().bitcast(BF16)[:, 0:512]
            tps = tps_flat.rearrange("p (qb q) -> p qb q", qb=4)
            for qb in (2 * half, 2 * half + 1):
                nc.tensor.transpose(tps[:, qb, :], avn[:, qb, :], ident)
            dst = op_tiles[qc][:, m, half * 256:(half + 1) * 256]
            src = tps_flat[:, (2 * half) * 128:(2 * half) * 128 + 256]
            eng = copy_rr[engi % 2]
            if eng is nc.scalar:
                nc.scalar.activation(out=dst, in_=src, func=AF.Identity,
                                     bias=qkvb[:, 4 + m:5 + m])
            else:
                eng.tensor_scalar(out=dst, in0=src,
                                  scalar1=qkvb[:, 4 + m:5 + m],
                                  scalar2=None, op0=ALU.add)

        def emit_outproj(qc, m2, engi):
            ps = new_ops()
            nc.tensor.matmul(ps, lhsT=w8out[:, :, m2 * 128:(m2 + 1) * 128],
                             rhs=op_tiles[qc], start=True, stop=True,
                             perf_mode=PM.DoubleRow, skip_group_check=True)
            yt = drp.tile([128, 512], F32, tag="yt", name="yt", bufs=4)
            eng = copy_rr[engi % 2]
            if eng is nc.scalar:
                nc.scalar.activation(out=yt, in_=ps, func=AF.Copy)
            else:
                eng.tensor_copy(yt, ps)
            nc.sync.dma_start(
                out=y_out[m2 * 128:(m2 + 1) * 128,
                          qc * 512:(qc + 1) * 512],
                in_=yt)

        # Event-driven emission: per step (qc, j) emit scores+exp, then any
        # due deferred work (v/q proj early, AV batches, drains, outproj).
        events = {}     # step -> list of callables

        def at(step, fn):
            events.setdefault(step, []).append(fn)

        # remaining q projections early in the qc0 stream
        for i, (s, ncx) in enumerate([(s, ncx) for ncx in range(1, 4)
                                      for s in range(2)]):
            at(i, (lambda s=s, ncx=ncx, i=i:
                   kq_proj("q", s, ncx, i)))
        # v projections: block nb at step nb
        for nb in range(NKB):
            at(nb, (lambda nb=nb: v_proj(nb, nb)))

        rr = [0]

        def nrr():
            rr[0] += 1
            return rr[0]

        total_steps = QC * NKB
        for qc in range(QC):
            base = qc * NKB
            # m0 AV batches chase the exp stream with lag 2; jp0 cannot
            # start before the PREVIOUS qc's m1 pair has fully drained
            # (single av accumulator), which happens at step base+7.
            for jp in range(NJP):
                at(base + max(2 * jp + 3, 7), (lambda qc=qc, jp=jp:
                                               emit_av(qc, m=0, jp=jp)))
            # m0 drain_b after drain_a (emitted inside emit_av at jp7)
            at(base + NKB + 2, (lambda qc=qc: drain_b(qc, 0, 0, nrr())))
            at(base + NKB + 3, (lambda qc=qc: drain_b(qc, 0, 1, nrr())))
            # m1 AV burst after m0 drained: 2 jp per step
            for i in range(4):
                def m1burst(qc=qc, i=i):
                    emit_av(qc, 1, 2 * i)
                    emit_av(qc, 1, 2 * i + 1)
                at(base + NKB + 3 + i, m1burst)
            at(base + NKB + 8, (lambda qc=qc: drain_b(qc, 1, 0, nrr())))
            at(base + NKB + 9, (lambda qc=qc: drain_b(qc, 1, 1, nrr())))
            for m2 in range(4):
                at(base + NKB + 10 + m2,
                   (lambda qc=qc, m2=m2: emit_outproj(qc, m2, nrr())))

        max_step = max(events) + 1
        for step in range(max_step):
            if step < total_steps:
                qc, j = step // NKB, step % NKB
                emit_scores_exp(qc, j, 0)
                emit_scores_exp(qc, j, 1)
            for fn in events.get(step, ()):
                fn()

    nc.compile()
    return nc


_NC = None


def _get_nc():
    global _NC
    if _NC is None:
        _NC = _build()
    return _NC


def _gblk():
    g = np.zeros((128, 8), dtype=np.float32)
    for p in range(128):
        g[p, p // CPG] = 1.0
    return g


def kernel(x, gn_gamma, gn_beta, w_qkv, w_out, b_out, trace=False):
    E4N = ml_dtypes.float8_e4m3
    x = np.asarray(x, dtype=np.float32)
    w_qkv = np.asarray(w_qkv, np.float32)
    w_out = np.asarray(w_out, np.float32)
    gblk = _gblk()
    gbt = np.ascontiguousarray(gblk.T)
    gamma = np.asarray(gn_gamma, np.float32).reshape(C)
    beta = np.asarray(gn_beta, np.float32).reshape(C)
    gbo4 = np.zeros((128, 8), dtype=np.float32)
    for u in range(UN):
        gbo4[:, 2 * u] = gamma[u * 128:(u + 1) * 128]
        gbo4[:, 2 * u + 1] = beta[u * 128:(u + 1) * 128]
    ident = np.eye(128, dtype=np.float32).astype(ml_dtypes.bfloat16)

    nc = _get_nc()
    in_maps = []
    for core in range(8):
        b, hg = core // 2, core % 2
        # x8: [128, u, n] with channel c = u*128 + p
        x8 = np.ascontiguousarray(
            x[b].reshape(UN, 128, N).transpose(1, 0, 2)).astype(E4N)
        # wbf cols: [q_s0 | q_s1 | k_s0 | k_s1 | v], rows c = u*128+p
        hgr = slice(hg * 256, (hg + 1) * 256)
        wq = w_qkv[0:C][hgr].reshape(HL, 2, 32, C)      # [h, s, dd, c]
        wk = w_qkv[C:2 * C][hgr].reshape(HL, 2, 32, C)
        wv = w_qkv[2 * C:3 * C][hgr].reshape(HL, 64, C)  # [h, d, c]
        cols = np.concatenate([
            wq[:, 0].reshape(128, C), wq[:, 1].reshape(128, C),
            wk[:, 0].reshape(128, C), wk[:, 1].reshape(128, C),
            wv.reshape(256, C)], axis=0)                 # [768, C]
        wbf = np.ascontiguousarray(
            cols.T.reshape(UN, 128, 768).transpose(1, 0, 2)
        ).astype(ml_dtypes.bfloat16)
        # w8out[p, m, oc] = w_out[oc, hg*256 + m*128 + p]
        wo = w_out[:, hgr]                               # [C, 256]
        w8o = np.ascontiguousarray(
            wo.T.reshape(2, 128, C).transpose(1, 0, 2)).astype(E4N)
        in_maps.append({
            "x8": x8,
            "wbf": wbf,
            "wout8": w8o,
            "gbo": gbo4,
            "gblk": gblk,
            "gbt": gbt,
            "ident": ident,
        })
    res = run_bass_kernel_spmd(nc, in_maps, core_ids=list(range(8)),
                               trace=trace)
    y = np.empty((B, C, N), dtype=np.float32)
    bo = np.asarray(b_out, np.float32).reshape(C, 1)
    for b in range(B):
        y[b] = ((res.results[2 * b]["y"] + res.results[2 * b + 1]["y"])
                * np.float32(1.0 / AVS) + x[b] + bo)
    if trace:
        kernel.last_results = res
    return y
